# revision 22
# baseline (speedup 1.0000x reference)
"""Causal self-MQA kernel for Trainium2, sharded over 8 NeuronCores.

Problem: B=2, S=2048, D=2048, H=16 query heads, DH=128, single KV head,
GPT-NeoX RoPE, causal attention, fused q/kv/o projections.

Sharding: 8 cores = 2 batches x 4 head-groups (4 heads = 512 q-dims per
core). The tiny kv projection is replicated within a batch. Each core
computes a partial output [S, D] (its head-group's contribution through
the o-projection); the host sums the 4 partials per batch and adds
o_bias.

Precision strategy (harness gate is rel_err < 2e-2):
  - q/k/v projections run as fp8e4 DoubleRow matmuls (0.5 PE cycles/row,
    256-deep contraction) with hi+lo operand splitting done ON HOST:
    x*8 and W*64 are each split into (e4m3 hi, e4m3 lo of the residual);
    three DoubleRow passes (hi*hi + hi*lo + lo*hi) recover ~11-bit
    accuracy. The 1/512 dequant rides existing ACT scale slots for q/k;
    for v it is deferred through the (linear) attention + o-projection
    and folded into the output-copy scale.
  - attention (scores, exp, AV) and the o-projection run in bf16
    (1.0 PE cycles/row at any tile width, 2-4x faster DVE ops, half the
    DMA bytes).
  - softmax denominators: instead of a ones-vector PE matmul per k-block
    (which costs as many PE rows as the AV matmul itself), exp tiles are
    folded into a per-unit fp16 accumulator on the DVE (2-byte dtypes hit
    the DVE 2x/4x fast path) and ONE small fp16 matmul per (head,
    q-chunk) reduces it over partitions. fp16 range is safe: col sums of
    exp(scores) stay < ~4e3 << 65504 for this problem's score scale.
  - V projection is emitted TRANSPOSED (out [s, dh] directly) by using
    the x tiles as the stationary operand, so no PE transposes of V are
    needed; its bias (pre-scaled by 512) is added on the DVE during the
    PSUM->SBUF copy.

Layouts keep the feature dim on partitions so no activation transpose is
needed anywhere:
  qT[dh, s] = wqT.T @ xT          (DoubleRow: lhsT = wq tiles, rhs = x)
  rotate_half(q) = swap_matrix @ qT   (PE matmul; sign folded into sinT)
  scoresT[k, q] = k_ropeT(dh,k).T @ q_ropeT(dh,q)
  softmax over k = PARTITION dim: no max-subtraction (|scores| < ~7),
    causal mask as a 0/1 bf16 multiply on the exp tile (diagonal blocks
    only), denominators via the DVE fold + fp16 matmul described above,
    reciprocal on DVE, partition-broadcast on GpSimd.
  attnT[dh, q] += v_nat(k,dh).T @ expT(k,q)   accumulated over k blocks
  out_part[s_blk, d] = attnT_blocks.T @ woT tiles   (bf16)
"""

import os
import sys

import numpy as np
import ml_dtypes

for _p in ("/opt/trn_rl_repo", "/root/.axon_site/_ro/trn_rl_repo"):
    if os.path.isdir(_p) and _p not in sys.path:
        sys.path.insert(0, _p)

import concourse.bass as bass  # noqa: E402,F401
import concourse.mybir as mybir  # noqa: E402
import concourse.tile as tile  # noqa: E402
from concourse import bacc  # noqa: E402
from concourse.bass_utils import run_bass_kernel_spmd  # noqa: E402

B, S, D = 2, 2048, 2048
H, DH = 16, 128
G = 4          # head groups (cores per batch)
HPG = 4        # heads per group
C = HPG * DH   # 512 output dims per group
SC = 256       # projection s-chunk width
NSC = S // SC  # 8
KT = D // 128  # 16 contraction tiles (8 DoubleRow pairs)
KP = KT // 2   # 8 fp8 DoubleRow k-tile pairs
QC = 512       # attention q-chunk width
NQC = S // QC  # 4
NSB = S // 128  # 16 s-blocks

SX = 8.0       # host scale on x before fp8 split
SW = 64.0      # host scale on weights before fp8 split
DEQ = 1.0 / (SX * SW)

F32 = mybir.dt.float32
BF16 = mybir.dt.bfloat16
F16 = mybir.dt.float16
F8 = mybir.dt.float8e4
AF = mybir.ActivationFunctionType
OP = mybir.AluOpType
DR = mybir.MatmulPerfMode.DoubleRow

_NC_CACHE = {}


def build_nc():
    nc = bacc.Bacc("TRN2", target_bir_lowering=False, debug=False)

    # x / weight tensors arrive in SBUF-native p-major layout (partition
    # first, then contiguous per-partition bytes) so every DMA line is
    # >= 4KB and runs at full HBM rate.
    xhi = nc.dram_tensor("xhi", [128, NSC, KT, SC], F8, kind="ExternalInput").ap()
    xlo = nc.dram_tensor("xlo", [128, NSC, KT, SC], F8, kind="ExternalInput").ap()
    wqhi = nc.dram_tensor("wqhi", [128, HPG, KT, DH], F8, kind="ExternalInput").ap()
    wqlo = nc.dram_tensor("wqlo", [128, HPG, KT, DH], F8, kind="ExternalInput").ap()
    wkvhi = nc.dram_tensor("wkvhi", [128, KT, 2 * DH], F8, kind="ExternalInput").ap()
    wkvlo = nc.dram_tensor("wkvlo", [128, KT, 2 * DH], F8, kind="ExternalInput").ap()
    woT = nc.dram_tensor("woT", [C, D], BF16, kind="ExternalInput").ap()
    qb = nc.dram_tensor("qb", [DH, HPG], F32, kind="ExternalInput").ap()
    kb = nc.dram_tensor("kb", [DH, 1], F32, kind="ExternalInput").ap()
    vbias = nc.dram_tensor("vbias", [128, DH], BF16, kind="ExternalInput").ap()
    cost = nc.dram_tensor("cost", [DH, S], BF16, kind="ExternalInput").ap()
    sint = nc.dram_tensor("sint", [DH, S], BF16, kind="ExternalInput").ap()
    mask01 = nc.dram_tensor("mask01", [128, 128], BF16, kind="ExternalInput").ap()
    swap = nc.dram_tensor("swap", [128, 128], BF16, kind="ExternalInput").ap()
    out_p = nc.dram_tensor("out_p", [S, D], BF16, kind="ExternalOutput").ap()

    with tile.TileContext(nc) as tc:
        _body(nc, tc, xhi, xlo, wqhi, wqlo, wkvhi, wkvlo, woT, qb, kb, vbias,
              cost, sint, mask01, swap, out_p)
    nc.compile()
    return nc


def _body(nc, tc, xhi, xlo, wqhi, wqlo, wkvhi, wkvlo, woT, qb, kb, vbias,
          cost, sint, mask01, swap, out_p):
    consts = tc.alloc_tile_pool(name="consts", bufs=1)
    sb = tc.alloc_tile_pool(name="sb", bufs=2)
    psum = tc.alloc_tile_pool(name="psum", bufs=1, space="PSUM")

    cost_sb = consts.tile([DH, S], BF16, tag="cost", name="cost")
    sint_sb = consts.tile([DH, S], BF16, tag="sint", name="sint")
    mask_sb = consts.tile([128, 128], BF16, tag="mask", name="mask")
    swap_sb = consts.tile([128, 128], BF16, tag="swap", name="swap")
    qb_sb = consts.tile([DH, HPG], F32, tag="qb", name="qb")
    kb_sb = consts.tile([DH, 1], F32, tag="kb", name="kb")
    vb_sb = consts.tile([128, DH], BF16, tag="vb", name="vb")
    wq_hi = consts.tile([128, HPG, KT, DH], F8, tag="wqhi", name="wqhi")
    wq_lo = consts.tile([128, HPG, KT, DH], F8, tag="wqlo", name="wqlo")
    wkv_hi = consts.tile([128, KT, 2 * DH], F8, tag="wkvhi", name="wkvhi")
    wkv_lo = consts.tile([128, KT, 2 * DH], F8, tag="wkvlo", name="wkvlo")

    def load_critical_weights():
        # sync-queue, in first-need order after chunk 0's xh/wkv_hi (issued
        # before this): k-rope needs kb/swap, q needs wq head 0 first
        nc.sync.dma_start(vb_sb, vbias)
        nc.sync.dma_start(kb_sb, kb)
        nc.sync.dma_start(swap_sb, swap)
        nc.sync.dma_start(qb_sb, qb)
        # rope tables issue from the ACT queue so they interleave with the
        # weight stream instead of queuing behind it
        nc.scalar.dma_start(cost_sb, cost)
        nc.scalar.dma_start(sint_sb, sint)

    def load_wq_head(h):
        # head-major layout: per-head slices are contiguous (full DMA rate)
        # so q head h can start as soon as its own weights land
        nc.sync.dma_start(wq_hi[:, h, :, :], wqhi[:, h, :, :])
        nc.sync.dma_start(wq_lo[:, h, :, :], wqlo[:, h, :, :])

    def load_rest_of_weights():
        # issued from the ACT queue AFTER its first op, keeping these off
        # the HBM port during the critical startup window
        nc.scalar.dma_start(mask_sb, mask01)

    # ---- persistent activations ----
    q_rope = [consts.tile([DH, S], BF16, tag=f"qrope{h}", name=f"qrope{h}")
              for h in range(HPG)]
    k_rope = consts.tile([DH, S], BF16, tag="krope", name="krope")
    v_nat = consts.tile([128, NSB, DH], BF16, tag="vnat", name="vnat")
    attn = [consts.tile([DH, S], BF16, tag=f"attn{h}", name=f"attn{h}")
            for h in range(HPG)]

    # ================= phase 1: q/kv projections + RoPE =================
    def rope(dst, ps, bias_col, ssl):
        """dst[:, ssl] = rope(ps*DEQ + bias).

        raw = ps*DEQ + bias (ACT, to SBUF bf16); rot = swap @ raw (PE);
        dst = raw*cos (DVE) + rot*sin_signed (DVE mul + GpSimd add).
        """
        raw = sb.tile([128, SC], BF16, tag="qraw", name="qraw", bufs=4)
        nc.scalar.activation(raw, ps, AF.Identity, bias=bias_col,
                             scale=DEQ)
        rot = psum.tile([128, SC], F32, tag="score", name="rotps", bufs=5)
        nc.tensor.matmul(rot, swap_sb, raw, start=True, stop=True)
        tmp = sb.tile([128, SC], BF16, tag="ropetmp", name="ropetmp",
                      bufs=2)
        nc.vector.tensor_mul(dst[:, ssl], raw, cost_sb[:, ssl])
        nc.vector.tensor_mul(tmp, rot, sint_sb[:, ssl])
        nc.gpsimd.tensor_add(dst[:, ssl], dst[:, ssl], tmp)

    def acc3(ps, lhs_hi, lhs_lo, rhs_hi, rhs_lo):
        """ps += lhsT.T @ rhs over KP DoubleRow pairs, 3 hi/lo passes."""
        passes = [(lhs_hi, rhs_hi), (lhs_hi, rhs_lo), (lhs_lo, rhs_hi)]
        n = len(passes) * KP
        i = 0
        for lh, rh in passes:
            for t in range(KP):
                tsl = slice(2 * t, 2 * t + 2)
                nc.tensor.matmul(ps, lh(tsl), rh(tsl),
                                 start=(i == 0), stop=(i == n - 1),
                                 perf_mode=DR)
                i += 1

    # v/k projections run LEAD chunks ahead of the q projections, so the
    # startup only has to stream x+wkv before the PE starts; the bigger wq
    # tensors stream per-head in the gaps between x chunk DMAs.
    LEAD = 3
    xtiles = {}

    def issue_x(sc):
        xh = sb.tile([128, KT, SC], F8, tag="xhi", name="xhi", bufs=LEAD + 2)
        xl = sb.tile([128, KT, SC], F8, tag="xlo", name="xlo", bufs=LEAD + 2)
        nc.sync.dma_start(xh, xhi[:, sc, :, :])
        if sc == 0:
            nc.sync.dma_start(wkv_hi, wkvhi)
        nc.sync.dma_start(xl, xlo[:, sc, :, :])
        if sc == 0:
            nc.sync.dma_start(wkv_lo, wkvlo)
            load_critical_weights()
        elif sc <= 2:
            load_wq_head(2 * sc - 2)
            load_wq_head(2 * sc - 1)
        xtiles[sc] = (xh, xl)

    def vk_chunk(sc):
        if sc not in xtiles:
            issue_x(sc)
        xh, xl = xtiles[sc]
        # v (transposed: out [s, dh])
        for j in range(SC // 128):
            jsl = slice(j * 128, (j + 1) * 128)
            vps = psum.tile([128, DH], F32, tag="av", name="vproj", bufs=3)
            acc3(vps,
                 lambda tsl, jsl=jsl: xh[:, tsl, jsl],
                 lambda tsl, jsl=jsl: xl[:, tsl, jsl],
                 lambda tsl: wkv_hi[:, tsl, DH:2 * DH],
                 lambda tsl: wkv_lo[:, tsl, DH:2 * DH])
            nc.vector.tensor_add(v_nat[:, sc * (SC // 128) + j, :], vps,
                                 vb_sb)
        if sc == 0:
            load_rest_of_weights()
        # k
        ps = psum.tile([128, SC], F32, tag="av", name="proj", bufs=3)
        acc3(ps,
             lambda tsl: wkv_hi[:, tsl, 0:DH],
             lambda tsl: wkv_lo[:, tsl, 0:DH],
             lambda tsl: xh[:, tsl, :],
             lambda tsl: xl[:, tsl, :])
        rope(k_rope, ps, kb_sb[:, 0:1], slice(sc * SC, (sc + 1) * SC))

    def q_chunk(sc):
        xh, xl = xtiles.pop(sc)
        for h in range(HPG):
            csl = slice(h * DH, (h + 1) * DH)
            ps = psum.tile([128, SC], F32, tag="av", name="proj", bufs=3)
            acc3(ps,
                 lambda tsl, h=h: wq_hi[:, h, tsl, :],
                 lambda tsl, h=h: wq_lo[:, h, tsl, :],
                 lambda tsl, xh=xh: xh[:, tsl, :],
                 lambda tsl, xl=xl: xl[:, tsl, :])
            rope(q_rope[h], ps, qb_sb[:, h:h + 1],
                 slice(sc * SC, (sc + 1) * SC))

    for i in range(NSC + LEAD):
        if i < NSC:
            vk_chunk(i)
        if i >= LEAD:
            q_chunk(i - LEAD)

    # ====== phases 2+3: causal attention (qc outer, head inner) with the
    # o-projection for q-chunk qc-1 interleaved into qc's attention, so the
    # 8.4 MB of output DMA spreads across the whole attention phase ======
    woT_r = woT.rearrange("(c p) n -> p c n", p=128)
    out_pr = out_p.rearrange("(sb p) n -> p sb n", p=128)

    wo = consts.tile([128, HPG, D], BF16, tag="wo", name="wo")
    nc.scalar.dma_start(wo, woT_r)

    # One oproj "group" = 4 accumulated matmuls for one (s-block, 512-wide d
    # chunk) + a PSUM->SBUF copy; DMA fires once both halves of an osb pair
    # are copied. Groups are queued and drained ONE PER ATTENTION TILE so
    # PSUM/copy/DMA pressure spreads evenly instead of bursting.
    oproj_pending = []
    oproj_osb = {}

    def oproj_enqueue(qc, min_i):
        for dc in range(4):
            for pair in range(2):
                for j in range(2):
                    oproj_pending.append((min_i, (qc, dc, pair, j)))

    def oproj_group(qc, dc, pair, j):
        dsl = slice(dc * 512, (dc + 1) * 512)
        if j == 0:
            osb = sb.tile([128, 2, 512], BF16, tag="osb", name="osb",
                          bufs=4)
            oproj_osb[(qc, dc, pair)] = osb
        else:
            osb = oproj_osb.pop((qc, dc, pair))
        sblk = qc * 4 + pair * 2 + j
        op = psum.tile([128, 512], F32, tag="score", name="oproj",
                       bufs=5)
        for c in range(HPG):
            nc.tensor.matmul(
                op, attn[c][:, sblk * 128:(sblk + 1) * 128],
                wo[:, c, dsl], start=(c == 0),
                stop=(c == HPG - 1))
        # 1/512 dequant of the v path rides the copy here
        if (sblk + dc) % 2 == 0:
            nc.scalar.activation(osb[:, j, :], op, AF.Copy,
                                 scale=DEQ)
        else:
            nc.vector.tensor_scalar_mul(osb[:, j, :], op, DEQ)
        if j == 1:
            nc.sync.dma_start(
                out_pr[:, qc * 4 + pair * 2:qc * 4 + pair * 2 + 2, dsl],
                osb)

    # Flat pipeline over ALL (qc, h, kj) regions — no drain at unit
    # boundaries. The softmax denominator (fold + partition-reduce +
    # reciprocal) rides the score/exp lookahead stream, so after a unit's
    # last AV matmul only the final normalize multiply remains before its
    # PSUM bank frees.
    units = [(h, qc) for qc in range(NQC) for h in range(HPG)]
    seq = []
    for ui, (h, qc) in enumerate(units):
        for kj in range(4 * qc + 4):
            seq.append((ui, kj))
    uacc = {}
    uav = {}
    ubcs = {}

    def emit_scores(ui, kj):
        h, qc = units[ui]
        st = max(0, kj * 128 - qc * QC)
        width = QC - st
        sp = psum.tile([128, QC], F32, tag="score", name="score", bufs=5)
        nc.tensor.matmul(
            sp[:, 0:width],
            k_rope[:, kj * 128:(kj + 1) * 128],
            q_rope[h][:, qc * QC + st:(qc + 1) * QC],
            start=True, stop=True)
        et = sb.tile([128, QC], BF16, tag="exp", name="exp", bufs=10)
        nc.scalar.activation(et[:, 0:width], sp[:, 0:width], AF.Exp)
        if kj >= 4 * qc:  # diagonal block: zero the upper triangle
            nc.gpsimd.tensor_mul(et[:, 0:128], et[:, 0:128], mask_sb)
        # denominator fold (fp16 accumulator, DVE 2-byte fast path)
        if kj == 0:
            acc = sb.tile([128, QC], F16, tag="acc", name="acc", bufs=3)
            uacc[ui] = acc
            nc.vector.tensor_copy(acc, et)
        else:
            acc = uacc[ui]
            nc.vector.tensor_add(acc[:, st:QC], acc[:, st:QC],
                                 et[:, 0:width])
        if kj == 4 * qc + 3:
            # GpSimd all-reduce over partitions: out has the per-q
            # denominator broadcast on every partition already.
            sums = sb.tile([128, QC], F32, tag="sums", name="sums", bufs=3)
            nc.gpsimd.partition_all_reduce(sums, acc, channels=128,
                                           reduce_op=bass.bass_isa.ReduceOp.add)
            bcs = sb.tile([128, QC], F32, tag="bcs", name="bcs", bufs=3)
            nc.vector.reciprocal(bcs, sums)
            ubcs[ui] = bcs
            del uacc[ui]
        return et, st, width

    def emit_av(ui, kj, ready, i):
        h, qc = units[ui]
        et, st, width = ready
        if kj == 0:
            uav[ui] = psum.tile([128, QC], F32, tag="av", name="av", bufs=3)
        att_ps = uav[ui]
        last = kj == 4 * qc + 3
        nc.tensor.matmul(
            att_ps[:, st:QC], v_nat[:, kj, :], et[:, 0:width],
            start=(kj == 0), stop=last, skip_group_check=True)
        if last:
            nc.vector.tensor_mul(attn[h][:, qc * QC:(qc + 1) * QC],
                                 att_ps, ubcs.pop(ui))
            del uav[ui]
            if h == HPG - 1:
                # cooldown: give the last head's normalize a few tiles of
                # headroom before oproj groups start reading attn
                oproj_enqueue(qc, i + 4)
        elif oproj_pending and oproj_pending[0][0] <= i:
            oproj_group(*oproj_pending.pop(0)[1])

    LOOKAHEAD = 8
    ready = {}
    for i in range(min(LOOKAHEAD, len(seq))):
        ready[i] = emit_scores(*seq[i])
    for i in range(len(seq)):
        nxt = i + LOOKAHEAD
        if nxt < len(seq):
            ready[nxt] = emit_scores(*seq[nxt])
        emit_av(*seq[i], ready.pop(i), i)
    while oproj_pending:
        oproj_group(*oproj_pending.pop(0)[1])

    psum.release()
    sb.release()
    consts.release()


def _host_tables():
    c4 = np.float32(1.0) / np.sqrt(np.sqrt(np.float32(DH)))
    inv_freq = (np.float32(1.0) / np.power(
        np.float32(10000.0),
        np.arange(0, DH, 2, dtype=np.float32) / np.float32(DH))).astype(np.float32)
    t = np.arange(S, dtype=np.float32)
    freqs = np.outer(t, inv_freq).astype(np.float32)          # [S, 64]
    emb = np.concatenate([freqs, freqs], axis=1)              # [S, 128]
    cost = (np.cos(emb).T * c4).astype(ml_dtypes.bfloat16)    # [128, S]
    sint = np.sin(emb).T * c4
    sint[0:64] *= np.float32(-1.0)                            # rotate_half sign
    sint = sint.astype(ml_dtypes.bfloat16)
    kq = np.arange(128, dtype=np.int64)
    mask01 = (kq[None, :] >= kq[:, None]).astype(ml_dtypes.bfloat16)  # [k,q]
    # swap[i, j] = 1 iff j == (i+64) % 128; symmetric, so it works as lhsT.
    swap = np.zeros((128, 128), np.float32)
    swap[kq, (kq + 64) % 128] = np.float32(1.0)
    return cost, sint, mask01, swap.astype(ml_dtypes.bfloat16)


def _split_fp8(a, scale):
    """scale*a -> (e4m3 hi, e4m3 lo) with lo = residual of hi."""
    f8 = ml_dtypes.float8_e4m3
    a = np.asarray(a, np.float32) * np.float32(scale)
    hi = a.astype(f8)
    lo = (a - hi.astype(np.float32)).astype(f8)
    return np.ascontiguousarray(hi), np.ascontiguousarray(lo)


def _pmajor_x(aT):
    """[D, S] -> SBUF-native [128, NSC, KT, SC]: p-major, 4KB DMA lines."""
    return np.ascontiguousarray(
        aT.reshape(KT, 128, NSC, SC).transpose(1, 2, 0, 3))


def _pmajor_w(wT):
    """[D, M] -> SBUF-native [128, KT, M]."""
    m = wT.shape[1]
    return np.ascontiguousarray(wT.reshape(KT, 128, m).transpose(1, 0, 2))


def _pmajor_wq(wT):
    """[D, C] -> head-major SBUF-native [128, HPG, KT, DH]."""
    return np.ascontiguousarray(
        wT.reshape(KT, 128, HPG, DH).transpose(1, 2, 0, 3))


def kernel(x, q_weight, q_bias, kv_weight, kv_bias, o_weight, o_bias):
    x = np.asarray(x, np.float32)
    q_weight = np.asarray(q_weight, np.float32)
    q_bias = np.asarray(q_bias, np.float32)
    kv_weight = np.asarray(kv_weight, np.float32)
    kv_bias = np.asarray(kv_bias, np.float32)
    o_weight = np.asarray(o_weight, np.float32)
    o_bias = np.asarray(o_bias, np.float32)

    if "nc" not in _NC_CACHE:
        _NC_CACHE["nc"] = build_nc()
    nc = _NC_CACHE["nc"]

    cost, sint, mask01, swap = _host_tables()
    kb = kv_bias[:DH].reshape(DH, 1).astype(np.float32)
    # v bias, pre-scaled by SX*SW (the deferred dequant divides it back out)
    vbias = np.broadcast_to(kv_bias[DH:] * np.float32(SX * SW),
                            (128, DH)).astype(ml_dtypes.bfloat16)
    vbias = np.ascontiguousarray(vbias)

    xhis, xlos = [], []
    for b in range(B):
        hi, lo = _split_fp8(x[b].T, SX)
        xhis.append(_pmajor_x(hi))
        xlos.append(_pmajor_x(lo))
    wkvhi, wkvlo = _split_fp8(kv_weight.T, SW)
    wkvhi, wkvlo = _pmajor_w(wkvhi), _pmajor_w(wkvlo)

    in_maps = []
    for core in range(8):
        b, g = divmod(core, G)
        wqhi, wqlo = _split_fp8(q_weight[g * C:(g + 1) * C].T, SW)
        wqhi, wqlo = _pmajor_wq(wqhi), _pmajor_wq(wqlo)
        in_maps.append({
            "xhi": xhis[b],
            "xlo": xlos[b],
            "wqhi": wqhi,
            "wqlo": wqlo,
            "wkvhi": wkvhi,
            "wkvlo": wkvlo,
            "woT": np.ascontiguousarray(
                o_weight[:, g * C:(g + 1) * C].T.astype(ml_dtypes.bfloat16)),
            "qb": np.ascontiguousarray(
                q_bias[g * C:(g + 1) * C].reshape(HPG, DH).T),
            "kb": kb,
            "vbias": vbias,
            "cost": cost,
            "sint": sint,
            "mask01": mask01,
            "swap": swap,
        })

    res = run_bass_kernel_spmd(nc, in_maps, core_ids=list(range(8)))

    out = np.zeros((B, S, D), np.float32)
    for core in range(8):
        out[core // G] += res.results[core]["out_p"].astype(np.float32)
    out += o_bias[None, None, :]
    return out


# revision 23
# speedup vs baseline: 1.0016x; 1.0016x over previous
"""Causal self-MQA kernel for Trainium2, sharded over 8 NeuronCores.

Problem: B=2, S=2048, D=2048, H=16 query heads, DH=128, single KV head,
GPT-NeoX RoPE, causal attention, fused q/kv/o projections.

Sharding: 8 cores = 2 batches x 4 head-groups (4 heads = 512 q-dims per
core). The tiny kv projection is replicated within a batch. Each core
computes a partial output [S, D] (its head-group's contribution through
the o-projection); the host sums the 4 partials per batch and adds
o_bias.

Precision strategy (harness gate is rel_err < 2e-2):
  - q/k/v projections run as fp8e4 DoubleRow matmuls (0.5 PE cycles/row,
    256-deep contraction) with hi+lo operand splitting done ON HOST:
    x*8 and W*64 are each split into (e4m3 hi, e4m3 lo of the residual);
    three DoubleRow passes (hi*hi + hi*lo + lo*hi) recover ~11-bit
    accuracy. The 1/512 dequant rides existing ACT scale slots for q/k;
    for v it is deferred through the (linear) attention + o-projection
    and folded into the output-copy scale.
  - attention (scores, exp, AV) and the o-projection run in bf16
    (1.0 PE cycles/row at any tile width, 2-4x faster DVE ops, half the
    DMA bytes).
  - softmax denominators: instead of a ones-vector PE matmul per k-block
    (which costs as many PE rows as the AV matmul itself), exp tiles are
    folded into a per-unit fp16 accumulator on the DVE (2-byte dtypes hit
    the DVE 2x/4x fast path) and ONE small fp16 matmul per (head,
    q-chunk) reduces it over partitions. fp16 range is safe: col sums of
    exp(scores) stay < ~4e3 << 65504 for this problem's score scale.
  - V projection is emitted TRANSPOSED (out [s, dh] directly) by using
    the x tiles as the stationary operand, so no PE transposes of V are
    needed; its bias (pre-scaled by 512) is added on the DVE during the
    PSUM->SBUF copy.

Layouts keep the feature dim on partitions so no activation transpose is
needed anywhere:
  qT[dh, s] = wqT.T @ xT          (DoubleRow: lhsT = wq tiles, rhs = x)
  rotate_half(q) = swap_matrix @ qT   (PE matmul; sign folded into sinT)
  scoresT[k, q] = k_ropeT(dh,k).T @ q_ropeT(dh,q)
  softmax over k = PARTITION dim: no max-subtraction (|scores| < ~7),
    causal mask as a 0/1 bf16 multiply on the exp tile (diagonal blocks
    only), denominators via the DVE fold + fp16 matmul described above,
    reciprocal on DVE, partition-broadcast on GpSimd.
  attnT[dh, q] += v_nat(k,dh).T @ expT(k,q)   accumulated over k blocks
  out_part[s_blk, d] = attnT_blocks.T @ woT tiles   (bf16)
"""

import os
import sys

import numpy as np
import ml_dtypes

for _p in ("/opt/trn_rl_repo", "/root/.axon_site/_ro/trn_rl_repo"):
    if os.path.isdir(_p) and _p not in sys.path:
        sys.path.insert(0, _p)

import concourse.bass as bass  # noqa: E402,F401
import concourse.mybir as mybir  # noqa: E402
import concourse.tile as tile  # noqa: E402
from concourse import bacc  # noqa: E402
from concourse.bass_utils import run_bass_kernel_spmd  # noqa: E402

B, S, D = 2, 2048, 2048
H, DH = 16, 128
G = 4          # head groups (cores per batch)
HPG = 4        # heads per group
C = HPG * DH   # 512 output dims per group
SC = 256       # projection s-chunk width
NSC = S // SC  # 8
KT = D // 128  # 16 contraction tiles (8 DoubleRow pairs)
KP = KT // 2   # 8 fp8 DoubleRow k-tile pairs
QC = 512       # attention q-chunk width
NQC = S // QC  # 4
NSB = S // 128  # 16 s-blocks

SX = 8.0       # host scale on x before fp8 split
SW = 64.0      # host scale on weights before fp8 split
DEQ = 1.0 / (SX * SW)

F32 = mybir.dt.float32
BF16 = mybir.dt.bfloat16
F16 = mybir.dt.float16
F8 = mybir.dt.float8e4
AF = mybir.ActivationFunctionType
OP = mybir.AluOpType
DR = mybir.MatmulPerfMode.DoubleRow

_NC_CACHE = {}


def build_nc():
    nc = bacc.Bacc("TRN2", target_bir_lowering=False, debug=False)

    # x / weight tensors arrive in SBUF-native p-major layout (partition
    # first, then contiguous per-partition bytes) so every DMA line is
    # >= 4KB and runs at full HBM rate.
    xhi = nc.dram_tensor("xhi", [128, NSC, KT, SC], F8, kind="ExternalInput").ap()
    xlo = nc.dram_tensor("xlo", [128, NSC, KT, SC], F8, kind="ExternalInput").ap()
    wqhi = nc.dram_tensor("wqhi", [128, HPG, KT, DH], F8, kind="ExternalInput").ap()
    wqlo = nc.dram_tensor("wqlo", [128, HPG, KT, DH], F8, kind="ExternalInput").ap()
    wkvhi = nc.dram_tensor("wkvhi", [128, KT, 2 * DH], F8, kind="ExternalInput").ap()
    wkvlo = nc.dram_tensor("wkvlo", [128, KT, 2 * DH], F8, kind="ExternalInput").ap()
    woT = nc.dram_tensor("woT", [C, D], BF16, kind="ExternalInput").ap()
    qb = nc.dram_tensor("qb", [DH, HPG], F32, kind="ExternalInput").ap()
    kb = nc.dram_tensor("kb", [DH, 1], F32, kind="ExternalInput").ap()
    vbias = nc.dram_tensor("vbias", [128, DH], BF16, kind="ExternalInput").ap()
    cost = nc.dram_tensor("cost", [DH, S], BF16, kind="ExternalInput").ap()
    sint = nc.dram_tensor("sint", [DH, S], BF16, kind="ExternalInput").ap()
    mask01 = nc.dram_tensor("mask01", [128, 128], BF16, kind="ExternalInput").ap()
    swap = nc.dram_tensor("swap", [128, 128], BF16, kind="ExternalInput").ap()
    out_p = nc.dram_tensor("out_p", [S, D], BF16, kind="ExternalOutput").ap()

    with tile.TileContext(nc) as tc:
        _body(nc, tc, xhi, xlo, wqhi, wqlo, wkvhi, wkvlo, woT, qb, kb, vbias,
              cost, sint, mask01, swap, out_p)
    nc.compile()
    return nc


def _body(nc, tc, xhi, xlo, wqhi, wqlo, wkvhi, wkvlo, woT, qb, kb, vbias,
          cost, sint, mask01, swap, out_p):
    consts = tc.alloc_tile_pool(name="consts", bufs=1)
    sb = tc.alloc_tile_pool(name="sb", bufs=2)
    psum = tc.alloc_tile_pool(name="psum", bufs=1, space="PSUM")

    cost_sb = consts.tile([DH, S], BF16, tag="cost", name="cost")
    sint_sb = consts.tile([DH, S], BF16, tag="sint", name="sint")
    mask_sb = consts.tile([128, 128], BF16, tag="mask", name="mask")
    swap_sb = consts.tile([128, 128], BF16, tag="swap", name="swap")
    qb_sb = consts.tile([DH, HPG], F32, tag="qb", name="qb")
    kb_sb = consts.tile([DH, 1], F32, tag="kb", name="kb")
    vb_sb = consts.tile([128, DH], BF16, tag="vb", name="vb")
    wq_hi = consts.tile([128, HPG, KT, DH], F8, tag="wqhi", name="wqhi")
    wq_lo = consts.tile([128, HPG, KT, DH], F8, tag="wqlo", name="wqlo")
    wkv_hi = consts.tile([128, KT, 2 * DH], F8, tag="wkvhi", name="wkvhi")
    wkv_lo = consts.tile([128, KT, 2 * DH], F8, tag="wkvlo", name="wkvlo")

    def load_critical_weights():
        # sync-queue, in first-need order after chunk 0's xh/wkv_hi (issued
        # before this): k-rope needs kb/swap, q needs wq head 0 first
        nc.sync.dma_start(vb_sb, vbias)
        nc.sync.dma_start(kb_sb, kb)
        nc.sync.dma_start(swap_sb, swap)
        nc.sync.dma_start(qb_sb, qb)
        # rope tables issue from the ACT queue so they interleave with the
        # weight stream instead of queuing behind it
        nc.scalar.dma_start(cost_sb, cost)
        nc.scalar.dma_start(sint_sb, sint)

    def load_wq_head(h):
        # head-major layout: per-head slices are contiguous (full DMA rate)
        # so q head h can start as soon as its own weights land
        nc.sync.dma_start(wq_hi[:, h, :, :], wqhi[:, h, :, :])
        nc.sync.dma_start(wq_lo[:, h, :, :], wqlo[:, h, :, :])

    def load_rest_of_weights():
        # issued from the ACT queue AFTER its first op, keeping these off
        # the HBM port during the critical startup window
        nc.scalar.dma_start(mask_sb, mask01)

    # ---- persistent activations ----
    q_rope = [consts.tile([DH, S], BF16, tag=f"qrope{h}", name=f"qrope{h}")
              for h in range(HPG)]
    k_rope = consts.tile([DH, S], BF16, tag="krope", name="krope")
    v_nat = consts.tile([128, NSB, DH], BF16, tag="vnat", name="vnat")
    attn = [consts.tile([DH, S], BF16, tag=f"attn{h}", name=f"attn{h}")
            for h in range(HPG)]

    # ================= phase 1: q/kv projections + RoPE =================
    def rope(dst, ps, bias_col, ssl):
        """dst[:, ssl] = rope(ps*DEQ + bias).

        raw = ps*DEQ + bias (ACT, to SBUF bf16); rot = swap @ raw (PE);
        dst = raw*cos (DVE) + rot*sin_signed (DVE mul + GpSimd add).
        """
        raw = sb.tile([128, SC], BF16, tag="qraw", name="qraw", bufs=4)
        nc.scalar.activation(raw, ps, AF.Identity, bias=bias_col,
                             scale=DEQ)
        rot = psum.tile([128, SC], F32, tag="score", name="rotps", bufs=5)
        nc.tensor.matmul(rot, swap_sb, raw, start=True, stop=True)
        tmp = sb.tile([128, SC], BF16, tag="ropetmp", name="ropetmp",
                      bufs=2)
        nc.vector.tensor_mul(dst[:, ssl], raw, cost_sb[:, ssl])
        nc.vector.tensor_mul(tmp, rot, sint_sb[:, ssl])
        nc.gpsimd.tensor_add(dst[:, ssl], dst[:, ssl], tmp)

    def acc3(ps, lhs_hi, lhs_lo, rhs_hi, rhs_lo):
        """ps += lhsT.T @ rhs over KP DoubleRow pairs, 3 hi/lo passes."""
        passes = [(lhs_hi, rhs_hi), (lhs_hi, rhs_lo), (lhs_lo, rhs_hi)]
        n = len(passes) * KP
        i = 0
        for lh, rh in passes:
            for t in range(KP):
                tsl = slice(2 * t, 2 * t + 2)
                nc.tensor.matmul(ps, lh(tsl), rh(tsl),
                                 start=(i == 0), stop=(i == n - 1),
                                 perf_mode=DR)
                i += 1

    # v/k projections run LEAD chunks ahead of the q projections, so the
    # startup only has to stream x+wkv before the PE starts; the bigger wq
    # tensors stream per-head in the gaps between x chunk DMAs.
    LEAD = 3
    xtiles = {}

    def issue_x(sc):
        xh = sb.tile([128, KT, SC], F8, tag="xhi", name="xhi", bufs=LEAD + 2)
        xl = sb.tile([128, KT, SC], F8, tag="xlo", name="xlo", bufs=LEAD + 2)
        nc.sync.dma_start(xh, xhi[:, sc, :, :])
        if sc == 0:
            nc.sync.dma_start(wkv_hi, wkvhi)
        nc.sync.dma_start(xl, xlo[:, sc, :, :])
        if sc == 0:
            nc.sync.dma_start(wkv_lo, wkvlo)
            load_critical_weights()
        elif sc <= 2:
            load_wq_head(2 * sc - 2)
            load_wq_head(2 * sc - 1)
        xtiles[sc] = (xh, xl)

    def vk_chunk(sc):
        if sc not in xtiles:
            issue_x(sc)
        xh, xl = xtiles[sc]
        # v (transposed: out [s, dh])
        for j in range(SC // 128):
            jsl = slice(j * 128, (j + 1) * 128)
            vps = psum.tile([128, DH], F32, tag="av", name="vproj", bufs=3)
            acc3(vps,
                 lambda tsl, jsl=jsl: xh[:, tsl, jsl],
                 lambda tsl, jsl=jsl: xl[:, tsl, jsl],
                 lambda tsl: wkv_hi[:, tsl, DH:2 * DH],
                 lambda tsl: wkv_lo[:, tsl, DH:2 * DH])
            nc.vector.tensor_add(v_nat[:, sc * (SC // 128) + j, :], vps,
                                 vb_sb)
        if sc == 0:
            load_rest_of_weights()
        # k
        ps = psum.tile([128, SC], F32, tag="av", name="proj", bufs=3)
        acc3(ps,
             lambda tsl: wkv_hi[:, tsl, 0:DH],
             lambda tsl: wkv_lo[:, tsl, 0:DH],
             lambda tsl: xh[:, tsl, :],
             lambda tsl: xl[:, tsl, :])
        rope(k_rope, ps, kb_sb[:, 0:1], slice(sc * SC, (sc + 1) * SC))

    def q_chunk(sc):
        xh, xl = xtiles.pop(sc)
        for h in range(HPG):
            csl = slice(h * DH, (h + 1) * DH)
            ps = psum.tile([128, SC], F32, tag="av", name="proj", bufs=3)
            acc3(ps,
                 lambda tsl, h=h: wq_hi[:, h, tsl, :],
                 lambda tsl, h=h: wq_lo[:, h, tsl, :],
                 lambda tsl, xh=xh: xh[:, tsl, :],
                 lambda tsl, xl=xl: xl[:, tsl, :])
            rope(q_rope[h], ps, qb_sb[:, h:h + 1],
                 slice(sc * SC, (sc + 1) * SC))

    for i in range(NSC + LEAD):
        if i < NSC:
            vk_chunk(i)
        if i >= LEAD:
            q_chunk(i - LEAD)

    # ====== phases 2+3: causal attention (qc outer, head inner) with the
    # o-projection for q-chunk qc-1 interleaved into qc's attention, so the
    # 8.4 MB of output DMA spreads across the whole attention phase ======
    woT_r = woT.rearrange("(c p) n -> p c n", p=128)
    out_pr = out_p.rearrange("(sb p) n -> p sb n", p=128)

    wo = consts.tile([128, HPG, D], BF16, tag="wo", name="wo")
    for c in range(HPG):
        # four 0.5MB pieces: each DMA-engine hold is short, so x-chunk
        # streaming slots in between
        nc.scalar.dma_start(wo[:, c, :], woT_r[:, c, :])

    # One oproj "group" = 4 accumulated matmuls for one (s-block, 512-wide d
    # chunk) + a PSUM->SBUF copy; DMA fires once both halves of an osb pair
    # are copied. Groups are queued and drained ONE PER ATTENTION TILE so
    # PSUM/copy/DMA pressure spreads evenly instead of bursting.
    oproj_pending = []
    oproj_osb = {}

    def oproj_enqueue(qc, min_i):
        for dc in range(4):
            for pair in range(2):
                for j in range(2):
                    oproj_pending.append((min_i, (qc, dc, pair, j)))

    def oproj_group(qc, dc, pair, j):
        dsl = slice(dc * 512, (dc + 1) * 512)
        if j == 0:
            osb = sb.tile([128, 2, 512], BF16, tag="osb", name="osb",
                          bufs=8)
            oproj_osb[(qc, dc, pair)] = osb
        else:
            osb = oproj_osb.pop((qc, dc, pair))
        sblk = qc * 4 + pair * 2 + j
        op = psum.tile([128, 512], F32, tag="score", name="oproj",
                       bufs=5)
        for c in range(HPG):
            nc.tensor.matmul(
                op, attn[c][:, sblk * 128:(sblk + 1) * 128],
                wo[:, c, dsl], start=(c == 0),
                stop=(c == HPG - 1))
        # 1/512 dequant of the v path rides the copy here
        if (sblk + dc) % 2 == 0:
            nc.scalar.activation(osb[:, j, :], op, AF.Copy,
                                 scale=DEQ)
        else:
            nc.vector.tensor_scalar_mul(osb[:, j, :], op, DEQ)
        if j == 1:
            nc.sync.dma_start(
                out_pr[:, qc * 4 + pair * 2:qc * 4 + pair * 2 + 2, dsl],
                osb)

    # Flat pipeline over ALL (qc, h, kj) regions — no drain at unit
    # boundaries. The softmax denominator (fold + partition-reduce +
    # reciprocal) rides the score/exp lookahead stream, so after a unit's
    # last AV matmul only the final normalize multiply remains before its
    # PSUM bank frees.
    units = [(h, qc) for qc in range(NQC) for h in range(HPG)]
    seq = []
    for ui, (h, qc) in enumerate(units):
        for kj in range(4 * qc + 4):
            seq.append((ui, kj))
    uacc = {}
    uav = {}
    ubcs = {}

    def emit_scores(ui, kj):
        h, qc = units[ui]
        st = max(0, kj * 128 - qc * QC)
        width = QC - st
        sp = psum.tile([128, QC], F32, tag="score", name="score", bufs=5)
        nc.tensor.matmul(
            sp[:, 0:width],
            k_rope[:, kj * 128:(kj + 1) * 128],
            q_rope[h][:, qc * QC + st:(qc + 1) * QC],
            start=True, stop=True)
        et = sb.tile([128, QC], BF16, tag="exp", name="exp", bufs=10)
        nc.scalar.activation(et[:, 0:width], sp[:, 0:width], AF.Exp)
        if kj >= 4 * qc:  # diagonal block: zero the upper triangle
            nc.gpsimd.tensor_mul(et[:, 0:128], et[:, 0:128], mask_sb)
        # denominator fold (fp16 accumulator, DVE 2-byte fast path)
        if kj == 0:
            acc = sb.tile([128, QC], F16, tag="acc", name="acc", bufs=3)
            uacc[ui] = acc
            nc.vector.tensor_copy(acc, et)
        else:
            acc = uacc[ui]
            nc.vector.tensor_add(acc[:, st:QC], acc[:, st:QC],
                                 et[:, 0:width])
        if kj == 4 * qc + 3:
            # GpSimd all-reduce over partitions: out has the per-q
            # denominator broadcast on every partition already.
            sums = sb.tile([128, QC], F32, tag="sums", name="sums", bufs=3)
            nc.gpsimd.partition_all_reduce(sums, acc, channels=128,
                                           reduce_op=bass.bass_isa.ReduceOp.add)
            bcs = sb.tile([128, QC], F32, tag="bcs", name="bcs", bufs=3)
            nc.vector.reciprocal(bcs, sums)
            ubcs[ui] = bcs
            del uacc[ui]
        return et, st, width

    def emit_av(ui, kj, ready, i):
        h, qc = units[ui]
        et, st, width = ready
        if kj == 0:
            uav[ui] = psum.tile([128, QC], F32, tag="av", name="av", bufs=3)
        att_ps = uav[ui]
        last = kj == 4 * qc + 3
        nc.tensor.matmul(
            att_ps[:, st:QC], v_nat[:, kj, :], et[:, 0:width],
            start=(kj == 0), stop=last, skip_group_check=True)
        if last:
            nc.vector.tensor_mul(attn[h][:, qc * QC:(qc + 1) * QC],
                                 att_ps, ubcs.pop(ui))
            del uav[ui]
            if h == HPG - 1:
                # cooldown: give the last head's normalize a few tiles of
                # headroom before oproj groups start reading attn
                oproj_enqueue(qc, i + 4)
        elif oproj_pending and oproj_pending[0][0] <= i:
            oproj_group(*oproj_pending.pop(0)[1])

    LOOKAHEAD = 8
    ready = {}
    for i in range(min(LOOKAHEAD, len(seq))):
        ready[i] = emit_scores(*seq[i])
    for i in range(len(seq)):
        nxt = i + LOOKAHEAD
        if nxt < len(seq):
            ready[nxt] = emit_scores(*seq[nxt])
        emit_av(*seq[i], ready.pop(i), i)
    while oproj_pending:
        oproj_group(*oproj_pending.pop(0)[1])

    psum.release()
    sb.release()
    consts.release()


def _host_tables():
    c4 = np.float32(1.0) / np.sqrt(np.sqrt(np.float32(DH)))
    inv_freq = (np.float32(1.0) / np.power(
        np.float32(10000.0),
        np.arange(0, DH, 2, dtype=np.float32) / np.float32(DH))).astype(np.float32)
    t = np.arange(S, dtype=np.float32)
    freqs = np.outer(t, inv_freq).astype(np.float32)          # [S, 64]
    emb = np.concatenate([freqs, freqs], axis=1)              # [S, 128]
    cost = (np.cos(emb).T * c4).astype(ml_dtypes.bfloat16)    # [128, S]
    sint = np.sin(emb).T * c4
    sint[0:64] *= np.float32(-1.0)                            # rotate_half sign
    sint = sint.astype(ml_dtypes.bfloat16)
    kq = np.arange(128, dtype=np.int64)
    mask01 = (kq[None, :] >= kq[:, None]).astype(ml_dtypes.bfloat16)  # [k,q]
    # swap[i, j] = 1 iff j == (i+64) % 128; symmetric, so it works as lhsT.
    swap = np.zeros((128, 128), np.float32)
    swap[kq, (kq + 64) % 128] = np.float32(1.0)
    return cost, sint, mask01, swap.astype(ml_dtypes.bfloat16)


def _split_fp8(a, scale):
    """scale*a -> (e4m3 hi, e4m3 lo) with lo = residual of hi."""
    f8 = ml_dtypes.float8_e4m3
    a = np.asarray(a, np.float32) * np.float32(scale)
    hi = a.astype(f8)
    lo = (a - hi.astype(np.float32)).astype(f8)
    return np.ascontiguousarray(hi), np.ascontiguousarray(lo)


def _pmajor_x(aT):
    """[D, S] -> SBUF-native [128, NSC, KT, SC]: p-major, 4KB DMA lines."""
    return np.ascontiguousarray(
        aT.reshape(KT, 128, NSC, SC).transpose(1, 2, 0, 3))


def _pmajor_w(wT):
    """[D, M] -> SBUF-native [128, KT, M]."""
    m = wT.shape[1]
    return np.ascontiguousarray(wT.reshape(KT, 128, m).transpose(1, 0, 2))


def _pmajor_wq(wT):
    """[D, C] -> head-major SBUF-native [128, HPG, KT, DH]."""
    return np.ascontiguousarray(
        wT.reshape(KT, 128, HPG, DH).transpose(1, 2, 0, 3))


def kernel(x, q_weight, q_bias, kv_weight, kv_bias, o_weight, o_bias):
    x = np.asarray(x, np.float32)
    q_weight = np.asarray(q_weight, np.float32)
    q_bias = np.asarray(q_bias, np.float32)
    kv_weight = np.asarray(kv_weight, np.float32)
    kv_bias = np.asarray(kv_bias, np.float32)
    o_weight = np.asarray(o_weight, np.float32)
    o_bias = np.asarray(o_bias, np.float32)

    if "nc" not in _NC_CACHE:
        _NC_CACHE["nc"] = build_nc()
    nc = _NC_CACHE["nc"]

    cost, sint, mask01, swap = _host_tables()
    kb = kv_bias[:DH].reshape(DH, 1).astype(np.float32)
    # v bias, pre-scaled by SX*SW (the deferred dequant divides it back out)
    vbias = np.broadcast_to(kv_bias[DH:] * np.float32(SX * SW),
                            (128, DH)).astype(ml_dtypes.bfloat16)
    vbias = np.ascontiguousarray(vbias)

    xhis, xlos = [], []
    for b in range(B):
        hi, lo = _split_fp8(x[b].T, SX)
        xhis.append(_pmajor_x(hi))
        xlos.append(_pmajor_x(lo))
    wkvhi, wkvlo = _split_fp8(kv_weight.T, SW)
    wkvhi, wkvlo = _pmajor_w(wkvhi), _pmajor_w(wkvlo)

    in_maps = []
    for core in range(8):
        b, g = divmod(core, G)
        wqhi, wqlo = _split_fp8(q_weight[g * C:(g + 1) * C].T, SW)
        wqhi, wqlo = _pmajor_wq(wqhi), _pmajor_wq(wqlo)
        in_maps.append({
            "xhi": xhis[b],
            "xlo": xlos[b],
            "wqhi": wqhi,
            "wqlo": wqlo,
            "wkvhi": wkvhi,
            "wkvlo": wkvlo,
            "woT": np.ascontiguousarray(
                o_weight[:, g * C:(g + 1) * C].T.astype(ml_dtypes.bfloat16)),
            "qb": np.ascontiguousarray(
                q_bias[g * C:(g + 1) * C].reshape(HPG, DH).T),
            "kb": kb,
            "vbias": vbias,
            "cost": cost,
            "sint": sint,
            "mask01": mask01,
            "swap": swap,
        })

    res = run_bass_kernel_spmd(nc, in_maps, core_ids=list(range(8)))

    out = np.zeros((B, S, D), np.float32)
    for core in range(8):
        out[core // G] += res.results[core]["out_p"].astype(np.float32)
    out += o_bias[None, None, :]
    return out
